# revision 21
# baseline (speedup 1.0000x reference)
"""Linformer attention Trainium2 kernel.

Sharding: 8 cores = 4 batches x 2 head-groups (8 heads each).
Key fact: reference reshapes (B,H,N,d)->(B,N,C) WITHOUT head transpose, so
output row r depends only on head h = r//256.  Each core therefore produces
a fully independent slice final[b, 2048*g:(g+1)*2048, :] - no collectives.

Per-core DRAM layouts (host pre-arranges for big contiguous DMAs):
  xTr   [128, 8, 4096] bf16   xTr[p,ct,n]   = x[b][n, ct*128+p]
  wq    [128, 8, 1536] bf16   wq[p,ct,m]    = Wqkv[rows[m], ct*128+p]
                              (m: 0:512 q | 512:1024 k | 1024:1536 v)
  eTr   [8, 128, 8, 4, 256]   eTr[nb,p,h,s,lm] = E[g8+h][lm, nb*512+s*128+p]
  wp    [128, 8, 1024] bf16   wp[p,ci,co]   = Wproj[co, ci*128+p]
  bias  [128, 8]       f32    bias[p,co]    = bproj[co*128+p]
  outT  [8, 128, 8, 256] f32  outT[h,p,co,q] = out[b, g*2048+h*256+q, co*128+p]

Device phases (single NEFF, Tile framework, bf16 matmuls):
  1. per 512-col n-block: q[m,n] tiles -> qT resident; kv[n,m] tiles
     (streamed); klm/vlm accumulated in SBUF f32 via E^T matmuls.
  2. per head: dotT[lm,n] = klm^T q ; exp via ACT(scale=1/8) with a ones
     column in the y-matmul lhsT producing rowsum rows; rowsums for all 16
     j-groups batched into ONE dram write + ONE partition-broadcast read;
     y copied to SBUF so PSUM frees immediately; normalize; project; DMA out.
"""

import sys

sys.path.insert(0, "/opt/trn_rl_repo")

import numpy as np
import ml_dtypes
from contextlib import ExitStack

import concourse.bass as bass
import concourse.tile as tile
from concourse import bacc
from concourse import mybir
from concourse.bass_utils import run_bass_kernel_spmd
from concourse.masks import make_identity

B, N, C = 4, 4096, 1024
H, K_LM = 16, 256
D = C // H  # 64
HPC = 8  # heads per core
F32 = mybir.dt.float32
BF16 = mybir.dt.bfloat16
FE = mybir.ActivationFunctionType

N_BLK = 512
N_BLKS = N // N_BLK  # 8


def build_program():
    nc = bacc.Bacc("TRN2", target_bir_lowering=False, debug=False, num_devices=8)

    xTr = nc.dram_tensor("xTr", [128, 8, N], BF16, kind="ExternalInput").ap()
    wq_d = nc.dram_tensor("wq", [128, 8, 3 * D * HPC], BF16, kind="ExternalInput").ap()
    eTr = nc.dram_tensor("eTr", [N_BLKS, 128, HPC, 4, K_LM], BF16, kind="ExternalInput").ap()
    wp_d = nc.dram_tensor("wp", [128, 8, C], BF16, kind="ExternalInput").ap()
    bias_d = nc.dram_tensor("bias", [128, 8], F32, kind="ExternalInput").ap()
    outT = nc.dram_tensor("outT", [HPC, 128, 8, 256], F32, kind="ExternalOutput").ap()
    rs_scr = nc.dram_tensor("rs_scr", [HPC, 16, 256], F32).ap()
    ri_scr = nc.dram_tensor("ri_scr", [HPC, 16, 256], F32).ap()

    with tile.TileContext(nc) as tc, ExitStack() as ctx:
        singles = ctx.enter_context(tc.tile_pool(name="singles", bufs=1))
        qres = ctx.enter_context(tc.tile_pool(name="qres", bufs=1))

        ident = singles.tile([128, 128], BF16)
        make_identity(nc, ident)
        bias_sb = singles.tile([128, 8], F32)
        wp_sb = singles.tile([128, 8, C], BF16)
        klmb = singles.tile([128, HPC, K_LM], BF16)  # rows 0:64 kT | 64:128 vT
        klm_fix = singles.tile([128, HPC // 2, K_LM], BF16)

        qT = [qres.tile([128, N], BF16, tag=f"qT{i}", name=f"qT{i}") for i in range(4)]

        # ---------------- Phase 1: qkv + landmark projection ----------------
        with tc.tile_pool(name="wqp", bufs=1) as wqp, \
             tc.tile_pool(name="xtp", bufs=3) as xtp, \
             tc.tile_pool(name="etp", bufs=2) as etp, \
             tc.tile_pool(name="kvp", bufs=8) as kvp, \
             tc.tile_pool(name="klp", bufs=1) as klp, \
             tc.tile_pool(name="ps_q", bufs=2, space="PSUM") as ps_q, \
             tc.tile_pool(name="ps_kv", bufs=2, space="PSUM") as ps_kv, \
             tc.tile_pool(name="ps_lm", bufs=2, space="PSUM") as ps_lm:

            wq_sb = wqp.tile([128, 8, 3 * D * HPC], BF16)
            klm_sb = klp.tile([128, HPC, K_LM], F32)

            def emit_lm(nb, kvs, ets):
                # landmark accumulation per head (contract over this block's n)
                final = nb == N_BLKS - 1
                for h in range(HPC):
                    plm = ps_lm.tile([128, K_LM], F32, tag="plm")
                    for s in range(4):
                        nc.tensor.matmul(
                            plm,
                            kvs[s][:, h * 128 : (h + 1) * 128],
                            ets[:, h, s, :],
                            start=(s == 0),
                            stop=(s == 3),
                        )
                    if nb == 0:
                        nc.vector.tensor_copy(klm_sb[:, h, :], plm)
                    else:
                        nc.vector.tensor_add(klm_sb[:, h, :], klm_sb[:, h, :], plm)
                    if final:
                        # finalize this head immediately so phase 2 can start
                        # while later heads' landmarks still accumulate
                        nc.vector.tensor_copy(klmb[:, h, :], klm_sb[:, h, :])
                        if h % 2 == 1:
                            hh = h // 2
                            nc.sync.dma_start(
                                out=klm_fix[64:128, hh, :], in_=klmb[0:64, h, :]
                            )
                            nc.sync.dma_start(
                                out=klm_fix[0:64, hh, :], in_=klmb[64:128, h, :]
                            )

            prev_lm = None
            for nb in range(N_BLKS):
                nsl = bass.ts(nb, N_BLK)
                xts = xtp.tile([128, 8, N_BLK], BF16, tag="xt")
                nc.sync.dma_start(out=xts, in_=xTr[:, :, nsl])
                if nb == 0:
                    # split so the first q-matmul starts as soon as chunk 0
                    # lands instead of waiting for the whole 3MB weight load
                    for ct in range(8):
                        nc.sync.dma_start(
                            out=wq_sb[:, ct, :], in_=wq_d[:, ct, :]
                        )
                ets = etp.tile([128, HPC, 4, K_LM], BF16, tag="et")
                nc.sync.dma_start(out=ets, in_=eTr[nb])

                # q: out[m(128), n(512)] ; lhsT = wq col slice
                for mt in range(4):
                    pq = ps_q.tile([128, N_BLK], F32, tag="pq")
                    for ct in range(8):
                        nc.tensor.matmul(
                            pq,
                            wq_sb[:, ct, mt * 128 : (mt + 1) * 128],
                            xts[:, ct, :],
                            start=(ct == 0),
                            stop=(ct == 7),
                        )
                    nc.vector.tensor_copy(qT[mt][:, nsl], pq)

                # kv: out[n(128), m(1024)] ; lhsT = x tile col slice
                kvs = []
                for s in range(4):
                    kvt = kvp.tile([128, 2 * D * HPC], BF16, tag="kv")
                    for half in range(2):
                        pkv = ps_kv.tile([128, 512], F32, tag="pkv")
                        msl = bass.ds(512 + half * 512, 512)
                        for ct in range(8):
                            nc.tensor.matmul(
                                pkv,
                                xts[:, ct, s * 128 : (s + 1) * 128],
                                wq_sb[:, ct, msl],
                                start=(ct == 0),
                                stop=(ct == 7),
                            )
                        # interleave: kvt col = h*128 + half*64 + d
                        kvi = kvt.rearrange(
                            "p (h two d) -> p two h d", two=2, d=D
                        )[:, half, :, :]
                        nc.scalar.activation(
                            kvi, pkv.rearrange("p (h d) -> p h d", d=D), FE.Copy
                        )
                    kvs.append(kvt)

                # lm one block behind so PE never waits on the kv copies
                if prev_lm is not None:
                    emit_lm(*prev_lm)
                prev_lm = (nb, kvs, ets)
            emit_lm(*prev_lm)
            # phase-2 weights: loaded late so they don't compete with the
            # startup-critical x/wq DMAs
            nc.sync.dma_start(out=wp_sb, in_=wp_d)
            nc.sync.dma_start(out=bias_sb, in_=bias_d)

        # ---------------- Phase 2+3: attention + projection per head --------
        with tc.tile_pool(name="expp", bufs=4) as expp, \
             tc.tile_pool(name="vop", bufs=6) as vop, \
             tc.tile_pool(name="ysb", bufs=3) as ysb, \
             tc.tile_pool(name="rsp", bufs=1) as rsp, \
             tc.tile_pool(name="rs16p", bufs=3) as rs16p, \
             tc.tile_pool(name="rcp", bufs=2) as rcp, \
             tc.tile_pool(name="orp", bufs=24) as orp, \
             tc.tile_pool(name="tmp_p", bufs=4) as tmp_p, \
             tc.tile_pool(name="fop", bufs=2) as fop, \
             tc.tile_pool(name="ps_t", bufs=1, space="PSUM") as ps_t, \
             tc.tile_pool(name="ps_dot", bufs=3, space="PSUM") as ps_dot, \
             tc.tile_pool(name="ps_y", bufs=2, space="PSUM") as ps_y, \
             tc.tile_pool(name="ps_f", bufs=2, space="PSUM") as ps_f:

            def emit_proj(h, orts):
                # projection for this head's 256 output rows
                fo = fop.tile([128, 8, 256], F32, tag="fo")
                for co in range(8):
                    pf = ps_f.tile([128, 256], F32, tag="pf")
                    for ci in range(8):
                        nc.tensor.matmul(
                            pf,
                            wp_sb[:, ci, co * 128 : (co + 1) * 128],
                            orts[ci],
                            start=(ci == 0),
                            stop=(ci == 7),
                        )
                    nc.vector.tensor_scalar_add(
                        fo[:, co, :], pf, bias_sb[:, co : co + 1]
                    )
                nc.sync.dma_start(out=outT[h], in_=fo)

            prev_head = None
            for h in range(HPC):
                p64 = 64 * (h % 2)
                qh = qT[h // 2][p64 : p64 + 64, :]

                # vones: [lm(128), 65] = [vlm | 1] per lm-half
                vones = []
                for half in range(2):
                    vt = vop.tile([128, 65], BF16, tag="vones")
                    pt = ps_t.tile([128, 64], BF16, tag="pt")
                    if h % 2 == 0:
                        vsrc = klmb[64:128, h, half * 128 : (half + 1) * 128]
                        idn = ident[64:128, 64:128]
                    else:
                        vsrc = klm_fix[0:64, h // 2, half * 128 : (half + 1) * 128]
                        idn = ident[0:64, 0:64]
                    nc.tensor.transpose(pt, vsrc, idn)
                    nc.vector.tensor_copy(vt[:, 0:64], pt)
                    nc.vector.memset(vt[:, 64:65], 1.0)
                    vones.append(vt)

                # dotT + exp -> expT [128, 4096] per lm-half
                exps = []
                for half in range(2):
                    ex = expp.tile([128, N], BF16, tag="exp")
                    if h % 2 == 0:
                        klmh = klmb[0:64, h, half * 128 : (half + 1) * 128]
                    else:
                        klmh = klm_fix[
                            64:128, h // 2, half * 128 : (half + 1) * 128
                        ]
                    for nt in range(8):
                        pd = ps_dot.tile([128, 512], F32, tag="pd")
                        nc.tensor.matmul(
                            pd,
                            klmh,
                            qh[:, bass.ts(nt, 512)],
                            start=True,
                            stop=True,
                        )
                        nc.scalar.activation(
                            ex[:, bass.ts(nt, 512)], pd, FE.Exp, scale=0.125
                        )
                    exps.append(ex)

                # proj one head behind, emitted between dot and y so the PE
                # has work while ACT finishes this head's exp tiles
                if prev_head is not None:
                    emit_proj(*prev_head)

                # y + rowsum; strided rhs does the reshape.  Two j's share one
                # PSUM bank (even j cols 0:256, odd j cols 256:512) in a
                # single accumulation group of 4 matmuls.
                y_all = ysb.tile([128, N], BF16, tag="yall")
                rs_all = rsp.tile([128, N], F32, tag="rsall")
                for jp in range(8):
                    py = ps_y.tile([128, 512], F32, tag="py")
                    for sub in range(2):
                        j = 2 * jp + sub
                        csl = bass.ts(sub, 256)
                        for half in range(2):
                            rhs = exps[half].rearrange(
                                "p (q j) -> p j q", j=16
                            )[:, j, :]
                            nc.tensor.matmul(
                                py[0:65, csl],
                                vones[half],
                                rhs,
                                start=(sub == 0 and half == 0),
                                stop=(sub == 1 and half == 1),
                            )
                    nc.vector.tensor_copy(
                        y_all[0:64, bass.ts(jp, 512)], py[0:64, :]
                    )
                    # rowsum row copy on ACT (reads PSUM; gpsimd cannot)
                    nc.scalar.activation(
                        rs_all[64:65, bass.ts(jp, 512)], py[64:65, :], FE.Copy
                    )

                # rowsum reciprocal+broadcast: round-trip 1 reshapes the
                # 1x4096 row onto 16 partitions for a cheap reciprocal,
                # round-trip 2 partition-broadcasts the reciprocals
                nc.sync.dma_start(out=rs_scr[h], in_=rs_all[64:65, :])
                rs16 = rs16p.tile([128, 256], F32, tag="rs16")
                nc.sync.dma_start(out=rs16[0:16, :], in_=rs_scr[h])
                nc.vector.reciprocal_approx_fast(
                    out=rs16[0:16, :], in_=rs16[0:16, :]
                )
                nc.sync.dma_start(out=ri_scr[h], in_=rs16[0:16, :])
                rc = rcp.tile([128, 16, 256], F32, tag="rc")
                nc.sync.dma_start(
                    out=rc[0:64, :, :],
                    in_=ri_scr[h : h + 1].broadcast_to((64, 16, 256)),
                )

                # normalize, split across DVE (even j) and gpsimd (odd j);
                # odd-j goes to partitions 64:128 via DMA
                orts = [
                    orp.tile([128, 256], BF16, tag="ort", name=f"ort{h}_{i}")
                    for i in range(8)
                ]
                for j in range(16):
                    if j % 2 == 0:
                        nc.vector.tensor_mul(
                            orts[j // 2][0:64, :],
                            y_all[0:64, bass.ts(j, 256)],
                            rc[0:64, j, :],
                        )
                    else:
                        tmp = tmp_p.tile([128, 256], BF16, tag="tmp")
                        nc.gpsimd.tensor_mul(
                            tmp[0:64, :],
                            y_all[0:64, bass.ts(j, 256)],
                            rc[0:64, j, :],
                        )
                        nc.sync.dma_start(
                            out=orts[j // 2][64:128, :], in_=tmp[0:64, :]
                        )

                prev_head = (h, orts)
            emit_proj(*prev_head)

    nc.compile()
    return nc


_NC_CACHE = None


def make_in_maps(inputs):
    x = np.asarray(inputs["x"], dtype=np.float32)
    Wqkv = np.asarray(inputs["Wqkv"], dtype=np.float32)
    E = np.asarray(inputs["E"], dtype=np.float32)
    Wproj = np.asarray(inputs["Wproj"], dtype=np.float32)
    bproj = np.asarray(inputs["bproj"], dtype=np.float32)

    # wp[p, ci, co] = Wproj[co, ci*128+p]
    wp = np.ascontiguousarray(
        Wproj.reshape(C, 8, 128).transpose(2, 1, 0)
    ).astype(ml_dtypes.bfloat16)
    bias = np.ascontiguousarray(bproj.reshape(8, 128).T)

    in_maps = []
    for cid in range(8):
        b, g = cid // 2, cid % 2
        rows = np.concatenate(
            [
                np.arange(s * C + g * HPC * D, s * C + g * HPC * D + HPC * D)
                for s in range(3)
            ]
        )
        # xTr[p, ct, n] = x[b][n, ct*128+p]
        xTr = np.ascontiguousarray(
            x[b].reshape(N, 8, 128).transpose(2, 1, 0)
        ).astype(ml_dtypes.bfloat16)
        # wq[p, ct, m] = Wqkv[rows[m], ct*128+p]
        wq = np.ascontiguousarray(
            Wqkv[rows].reshape(3 * D * HPC, 8, 128).transpose(2, 1, 0)
        ).astype(ml_dtypes.bfloat16)
        # eTr[nb, p, h, s, lm] = E[g*8+h][lm, nb*512+s*128+p]
        Eg = E[g * HPC : (g + 1) * HPC]  # [8, 256, 4096]
        eTr = np.ascontiguousarray(
            Eg.reshape(HPC, K_LM, N_BLKS, 4, 128).transpose(2, 4, 0, 3, 1)
        ).astype(ml_dtypes.bfloat16)
        in_maps.append(
            {"xTr": xTr, "wq": wq, "eTr": eTr, "wp": wp, "bias": bias}
        )
    return in_maps


def kernel(x, Wqkv, E, Wproj, bproj, **_):
    global _NC_CACHE
    if _NC_CACHE is None:
        _NC_CACHE = build_program()
    nc = _NC_CACHE

    in_maps = make_in_maps(
        {"x": x, "Wqkv": Wqkv, "E": E, "Wproj": Wproj, "bproj": bproj}
    )

    res = run_bass_kernel_spmd(nc, in_maps, core_ids=list(range(8)))

    out = np.empty((B, N, C), dtype=np.float32)
    for cid in range(8):
        b, g = cid // 2, cid % 2
        # outT[h, p, co, q] -> rows h*256+q, cols co*128+p
        o = res.results[cid]["outT"].transpose(0, 3, 2, 1).reshape(2048, C)
        out[b, g * 2048 : (g + 1) * 2048, :] = o
    return out
